# revision 41
# baseline (speedup 1.0000x reference)
"""Multi-head attention (B=4, S=2048, D=1024, H=16) on 8 NeuronCores.

Sharding: core c -> (batch b = c//2, head-group g = c%2 of 8 heads).
Each core computes QKV projections for its 8 heads, causal attention, and a
row-sharded output projection partial; the host sums the two partials per
batch and adds the output bias (with the V-bias contribution bv @ wo.T
folded in on the host, so the device never sees bv).

Device kernel layout (cost-model-driven; matmul cost = out free size):
  * Q/K produced transposed (head-pair dim on partitions); score matmuls
    ST = K @ Q^T contract d_k=64 on partitions, two PE quadrants.
  * Scores/exp computed only on live columns [live0:512) per 128-k block.
  * exp on ScalarE via one strided (p, 2, w) activation per block (both
    heads); multiplicative 0/1 mask patterns (Pool engine) for partial
    blocks; no max-subtraction (logits ~N(0,1)).
  * AV is flipped: out[q;128, V|1;65] = pt[:, qtile]^T @ [V|ones], so the
    streamed free dim is 65 instead of 128 -> ~2x fewer PE cycles, and the
    ones column accumulates the softmax denominator per q on partitions.
  * Normalization = per-partition reciprocal (ACT exp(-ln d) on a strided
    [128, 8] view) + DVE tensor_scalar multiplies; the normalized [q, f]
    tile is PE-transposed back to [f, q] for the output projection.
  * Q/K biases folded into the PSUM evacuation as per-partition adds.
  * Output projection accumulates 4 head-pairs in PSUM and DMAs the fp32
    partial straight to DRAM.
"""

import numpy as np
import ml_dtypes
from contextlib import ExitStack

import concourse.bass as bass
import concourse.bacc as bacc
import concourse.tile as tile
from concourse import mybir
from concourse.bass_utils import run_bass_kernel_spmd
from concourse.masks import make_identity

F32 = mybir.dt.float32
BF16 = mybir.dt.bfloat16
BF = ml_dtypes.bfloat16

B, S, D, H, DK = 4, 2048, 1024, 16, 64
NCORES = 8
GH = 8            # heads per core
DL = GH * DK      # 512 local feature dims
NPAIR = 4         # local head pairs
NR = 4            # q ranges of 512
NKB = S // 128    # 16 k blocks
KTILES = D // 128  # 8 contraction tiles
EXP = mybir.ActivationFunctionType.Exp
LN = mybir.ActivationFunctionType.Ln
SCALE = 1.0 / np.sqrt(DK)


class BlockInfo:
    __slots__ = ("j", "live0", "live1", "pat", "mul0", "mul1")

    def __init__(self, j, live0, live1, pat, mul0, mul1):
        self.j, self.live0, self.live1 = j, live0, live1
        self.pat, self.mul0, self.mul1 = pat, mul0, mul1


def classify_mask(mask):
    """Classify (512 q x 128 k) blocks of the attention mask.

    Returns (live, patterns): live[r] is a list of BlockInfo for the k-blocks
    that have any attendable position; patterns is a list of (128, 512)
    float32 0/1 tiles (k on rows, q-local on cols), deduplicated.
    """
    live = []
    patterns = []
    index = {}
    for r in range(NR):
        row = []
        qs = mask[512 * r: 512 * (r + 1), :]
        for j in range(NKB):
            blk = qs[:, 128 * j: 128 * (j + 1)]       # (512 q, 128 k)
            if not blk.any():
                continue
            if blk.all():
                row.append(BlockInfo(j, 0, 512, None, 0, 0))
                continue
            bt = blk.T                                  # (128 k, 512 q)
            colfull = bt.all(axis=0)
            colany = bt.any(axis=0)
            liveidx = np.nonzero(colany)[0]
            live0, live1 = int(liveidx.min()), int(liveidx.max()) + 1
            nonfull = np.nonzero(~colfull[live0:live1])[0]
            if len(nonfull) == 0:
                row.append(BlockInfo(j, live0, live1, None, 0, 0))
                continue
            mul0 = live0 + int(nonfull.min())
            mul1 = live0 + int(nonfull.max()) + 1
            pat = bt[:, mul0:mul1].astype(np.float32)
            key = (mul1 - mul0, pat.tobytes())
            if key not in index:
                index[key] = len(patterns)
                padded = np.zeros((128, 512), np.float32)
                padded[:, : mul1 - mul0] = pat
                patterns.append(padded)
            row.append(BlockInfo(j, live0, live1, index[key], mul0, mul1))
        if not row:
            raise NotImplementedError(
                "a 512-row q range attends to nothing; fully-masked rows "
                "are not supported"
            )
        for t in range(4):
            if not any(bi.live0 < 128 * (t + 1) and bi.live1 > 128 * t
                       for bi in row):
                raise NotImplementedError(
                    "a 128-row q tile attends to nothing")
        live.append(row)
    if len(patterns) > 8:
        raise NotImplementedError(f"{len(patterns)} unique mask patterns")
    return live, patterns


def build_program(live, n_pat):
    nc = bacc.Bacc("TRN2", target_bir_lowering=False, debug=False,
                   num_devices=NCORES)

    xqt = nc.dram_tensor("xqt", [D, S], BF16, kind="ExternalInput").ap()
    xkt = nc.dram_tensor("xkt", [D, S], BF16, kind="ExternalInput").ap()
    xvt = nc.dram_tensor("xvt", [D, S], BF16, kind="ExternalInput").ap()
    wqt = nc.dram_tensor("wqt", [D, DL], BF16, kind="ExternalInput").ap()
    wkt = nc.dram_tensor("wkt", [D, DL], BF16, kind="ExternalInput").ap()
    wvt = nc.dram_tensor("wvt", [D, DL], BF16, kind="ExternalInput").ap()
    wot = nc.dram_tensor("wot", [DL, D], BF16, kind="ExternalInput").ap()
    # q/k biases pre-transposed on the host: [128 partition dims, NPAIR]
    bqp = nc.dram_tensor("bqp", [128, NPAIR], F32, kind="ExternalInput").ap()
    bkp = nc.dram_tensor("bkp", [128, NPAIR], F32, kind="ExternalInput").ap()
    patd = nc.dram_tensor("pats", [max(n_pat, 1), 128, 512], BF16,
                          kind="ExternalInput").ap()
    outp = nc.dram_tensor("outp", [S, D], F32, kind="ExternalOutput").ap()

    with tile.TileContext(nc) as tc, ExitStack() as ctx:
        emit(ctx, tc, nc, live, n_pat,
             xqt, xkt, xvt, wqt, wkt, wvt, wot, bqp, bkp, patd, outp)
    nc.compile()
    return nc


I32 = mybir.dt.int32
# fp32 fast-reciprocal seed in negative space:
# yn0_bits = (d_bits ^ -1) + RECIP_C2  ==  -(0x7EF311C3 - d_bits)
RECIP_C2 = -17624636


def emit(ctx, tc, nc, live, n_pat,
         xqt, xkt, xvt, wqt, wkt, wvt, wot, bqp_d, bkp_d, patd, outp):
    wpool = ctx.enter_context(tc.tile_pool(name="w", bufs=1))
    qkpool = ctx.enter_context(tc.tile_pool(name="qk", bufs=1))
    vpool = ctx.enter_context(tc.tile_pool(name="vp", bufs=1))
    rot = ctx.enter_context(tc.tile_pool(name="rot", bufs=3))
    xs = ctx.enter_context(tc.tile_pool(name="xs", bufs=2))
    ptp = ctx.enter_context(tc.tile_pool(name="ptp", bufs=26))
    anup = ctx.enter_context(tc.tile_pool(name="anup", bufs=4))
    ntp = ctx.enter_context(tc.tile_pool(name="ntp", bufs=8))
    outs = ctx.enter_context(tc.tile_pool(name="outs", bufs=2))

    stps = ctx.enter_context(tc.tile_pool(name="stps", bufs=2, space="PSUM"))
    avps = ctx.enter_context(tc.tile_pool(name="avps", bufs=2, space="PSUM"))
    pps = ctx.enter_context(tc.tile_pool(name="pps", bufs=2, space="PSUM"))

    # ---- resident tiles ----
    def load(name, dram, shape):
        t = wpool.tile(shape, BF16, tag=name, name=name)
        nc.gpsimd.dma_start(t[:], dram)
        return t

    bqp = wpool.tile([128, NPAIR], F32, tag="bqp", name="bqp")
    nc.gpsimd.dma_start(bqp[:], bqp_d)
    bkp = wpool.tile([128, NPAIR], F32, tag="bkp", name="bkp")
    nc.gpsimd.dma_start(bkp[:], bkp_d)
    pat_sb = [load(f"pat{i}", patd[i], [128, 512]) for i in range(n_pat)]
    ident = wpool.tile([128, 128], BF16, tag="ident")
    make_identity(nc, ident[:])

    def alloc(name, shape):
        return wpool.tile(shape, BF16, tag=name, name=name)

    wq_t = [alloc(f"wq{i}", [128, DL]) for i in range(KTILES)]
    wk_t = [alloc(f"wk{i}", [128, DL]) for i in range(KTILES)]
    wv_t = [alloc(f"wv{i}", [128, DL]) for i in range(KTILES)]
    wo_t = [alloc(f"wo{i}", [128, 512]) for i in range(2 * NPAIR)]

    def load_w():
        for i in range(KTILES):
            nc.gpsimd.dma_start(wq_t[i][:], wqt[128 * i:128 * (i + 1), :])
            nc.gpsimd.dma_start(wk_t[i][:], wkt[128 * i:128 * (i + 1), :])
            nc.gpsimd.dma_start(wv_t[i][:], wvt[128 * i:128 * (i + 1), :])
        for i in range(2 * NPAIR):
            nc.gpsimd.dma_start(
                wo_t[i][:], wot[128 * (i // 2):128 * (i // 2 + 1),
                                512 * (i % 2):512 * (i % 2 + 1)])

    # kt is fully resident (scores for range r read k columns 0..512(r+1));
    # qt/ot only hold the current 512-q range and rotate (bufs=2)
    kt_t = [qkpool.tile([128, S], BF16, tag=f"kt{hp}", name=f"kt{hp}")
            for hp in range(NPAIR)]
    va_t = [vpool.tile([128, GH * 65], BF16, tag=f"va{t}", name=f"va{t}")
            for t in range(NKB)]

    # ---- PE filler units (emitted between score blocks to fill the gaps
    # the in-order PE stream would otherwise spend stalled on exp) ----

    # Q/K projection of one (tensor, head pair) for 512-token chunk sc
    def qk_unit(x_c, w_t, b_sb, hp, dest, sc):
        ps = pps.tile([128, 512], F32, tag="pps", name="qkps")
        for kt in range(KTILES):
            nc.tensor.matmul(
                ps[:], w_t[kt][:, 128 * hp:128 * (hp + 1)],
                x_c[kt][:], start=(kt == 0), stop=(kt == KTILES - 1))
        if dest is not None:
            nc.vector.tensor_scalar_add(dest[:], ps[:], b_sb[:, hp:hp + 1])
        else:
            nc.vector.tensor_scalar_add(
                kt_t[hp][:, 512 * sc:512 * (sc + 1)], ps[:],
                b_sb[:, hp:hp + 1])

    dmaq = [nc.sync, nc.gpsimd]

    def qk_units_sc(sc, qt_r):
        xq_c, xk_c = [], []
        for i in range(KTILES):
            xq = xs.tile([128, 512], BF16, tag=f"xq{i}", name=f"xq{i}_{sc}")
            nc.sync.dma_start(
                xq[:], xqt[128 * i:128 * (i + 1), 512 * sc:512 * (sc + 1)])
            xq_c.append(xq)
            xk = xs.tile([128, 512], BF16, tag=f"xk{i}", name=f"xk{i}_{sc}")
            nc.sync.dma_start(
                xk[:], xkt[128 * i:128 * (i + 1), 512 * sc:512 * (sc + 1)])
            xk_c.append(xk)
        units = []
        for hp in range(NPAIR):
            units.append(lambda hp=hp: qk_unit(
                xq_c, wq_t, bqp, hp, qt_r[hp][:], sc))
            units.append(lambda hp=hp: qk_unit(
                xk_c, wk_t, bkp, hp, None, sc))
        return units

    # V projection of one 128-token tile (all 8 heads), ones-augmented
    def v_units(t0, t1):
        if t0 >= t1:
            return []
        assert t1 - t0 <= 4
        chunks = []
        for kt in range(KTILES):
            xt = xs.tile([128, 512], BF16, tag="xv", name=f"xv{t0}_{kt}",
                         bufs=9)
            nc.gpsimd.dma_start(
                xt[:, 0:128 * (t1 - t0)],
                xvt[128 * kt:128 * (kt + 1), 128 * t0:128 * t1])
            chunks.append(xt)

        def v_unit(t):
            o = 128 * (t - t0)
            ps = pps.tile([128, 512], F32, tag="pps", name="vps")
            for kt in range(KTILES):
                nc.tensor.matmul(ps[:], chunks[kt][:, o:o + 128],
                                 wv_t[kt][:], start=(kt == 0),
                                 stop=(kt == KTILES - 1))
            va = va_t[t].rearrange("p (h w) -> p h w", w=65)
            nc.vector.tensor_copy(
                va[:, :, 0:64], ps.rearrange("p (h w) -> p h w", w=64))
            nc.gpsimd.memset(va[:, :, 64:65], 1.0)

        return [lambda t=t: v_unit(t) for t in range(t0, t1)]

    # output projection partial for one (q-tile, 512-col half)
    def o_unit(ot_r, t, nh):
        ps = pps.tile([128, 512], F32, tag="pps", name="ops")
        for hp in range(NPAIR):
            nc.tensor.matmul(
                ps[:], ot_r[hp][:, 128 * (t % 4):128 * (t % 4 + 1)],
                wo_t[2 * hp + nh][:],
                start=(hp == 0), stop=(hp == NPAIR - 1))
        osb = outs.tile([128, 512], F32, tag="osb")
        nc.vector.tensor_copy(osb[:], ps[:])
        nc.sync.dma_start(
            outp[128 * t:128 * (t + 1), 512 * nh:512 * (nh + 1)], osb[:])

    # ---- attention phases ----
    def scores_phase(hp, r, qt_r, fill):
        qt, kt_ = qt_r[hp], kt_t[hp]
        pts = []
        for bi in live[r]:
            j, lo, hi = bi.j, bi.live0, bi.live1
            st = stps.tile([128, 1024], F32, tag="st")
            nc.tensor.matmul(
                st[:, lo:hi],
                kt_[0:64, 128 * j:128 * (j + 1)],
                qt[0:64, lo:hi],
                start=True, stop=True, tile_position=(0, 0))
            nc.tensor.matmul(
                st[:, 512 + lo:512 + hi],
                kt_[64:128, 128 * j:128 * (j + 1)],
                qt[64:128, lo:hi],
                start=True, stop=True, tile_position=(64, 0))
            pt = ptp.tile([128, 1024], BF16, tag="pt")
            st3 = st.rearrange("p (h w) -> p h w", w=512)
            pt3 = pt.rearrange("p (h w) -> p h w", w=512)
            nc.scalar.activation(pt3[:, :, lo:hi], st3[:, :, lo:hi],
                                 EXP, scale=float(SCALE))
            # zero dead columns inside covered 128-q tiles so the flipped AV
            # matmuls never read garbage (no-op for causal masks)
            plo, phi = (lo // 128) * 128, -(-hi // 128) * 128
            for h in range(2):
                if bi.pat is not None:
                    sl = pt[:, 512 * h + bi.mul0:512 * h + bi.mul1]
                    nc.gpsimd.tensor_mul(
                        sl, sl, pat_sb[bi.pat][:, 0:bi.mul1 - bi.mul0])
                if plo < lo:
                    nc.gpsimd.memset(pt[:, 512 * h + plo:512 * h + lo], 0.0)
                if hi < phi:
                    nc.gpsimd.memset(pt[:, 512 * h + hi:512 * h + phi], 0.0)
            pts.append((bi, pt, 0))
            if len(pts) % 2 == 0:
                fill(1)
        return pts

    def av_phase(hp, r, pts, ot_r):
        # flipped AV per 128-q tile: out [q, V|1] accumulated over k blocks
        anu = anup.tile([128, 520], F32, tag="anu", name=f"anu{hp}_{r}")
        for t in range(4):
            js_t = [(bi, pt, base) for bi, pt, base in pts
                    if bi.live0 < 128 * (t + 1) and bi.live1 > 128 * t]
            nj = len(js_t)
            for h in range(2):
                # own bank per accumulation group (start zeroes a whole bank)
                av = avps.tile([128, 65], F32, tag="av")
                hl = 2 * hp + h
                for ji, (bi, pt, base) in enumerate(js_t):
                    o0 = base + 512 * h + 128 * t
                    nc.tensor.matmul(
                        av[:], pt[:, o0:o0 + 128],
                        va_t[bi.j][:, 65 * hl:65 * (hl + 1)],
                        start=(ji == 0), stop=(ji == nj - 1))
                o = 65 * (2 * t + h)
                nc.vector.tensor_copy(anu[:, o:o + 65], av[:])
        # negative reciprocals of the 8 denominator columns on DVE:
        # bit-trick seed + 2 Newton iterations in negative space
        # (yn' = yn * (2 + d*yn)); avoids ACT table thrash from Ln
        dw = rot.tile([128, 16], F32, tag="dw")
        danu = anu.rearrange("p (x w) -> p x w", w=65)[:, :, 64:65]
        dw3 = dw.rearrange("p (x w) -> p x w", w=1)
        yn3, m3 = dw3[:, 0:8, :], dw3[:, 8:16, :]
        nc.vector.tensor_scalar(
            out=m3.bitcast(I32), in0=danu.bitcast(I32),
            scalar1=-1, scalar2=None, op0=mybir.AluOpType.bitwise_xor)
        nc.vector.tensor_scalar(
            out=yn3.bitcast(I32), in0=m3.bitcast(I32),
            scalar1=RECIP_C2, scalar2=None, op0=mybir.AluOpType.add)
        for _ in range(2):
            nc.vector.tensor_mul(m3, danu, yn3)
            nc.vector.scalar_tensor_tensor(
                out=yn3, in0=m3, scalar=2.0, in1=yn3,
                op0=mybir.AluOpType.add, op1=mybir.AluOpType.mult)
        # flip sign bits: yn -> +1/d
        nc.vector.tensor_scalar(
            out=yn3.bitcast(I32), in0=yn3.bitcast(I32),
            scalar1=-(1 << 31), scalar2=None,
            op0=mybir.AluOpType.bitwise_xor)
        # normalize, pack head pair into [q, 128], transpose back
        for t in range(4):
            nt = ntp.tile([128, 128], BF16, tag="nt")
            for h in range(2):
                o = 65 * (2 * t + h)
                nc.vector.tensor_scalar_mul(
                    nt[:, 64 * h:64 * (h + 1)], anu[:, o:o + 64],
                    dw[:, 2 * t + h:2 * t + h + 1])
            tp = pps.tile([128, 128], BF16, tag="pps", name="tp")
            nc.tensor.transpose(tp[:], nt[:], ident[:])
            nc.vector.tensor_copy(
                ot_r[hp][:, 128 * t:128 * (t + 1)], tp[:])

    # ---- schedule: r-outer, hp-inner; qk(r+1)/v(r+1)/o-proj(r-1) are
    # emitted as fillers inside the score sections ----
    need_vt = [max(bi.j for bi in live[r]) + 1 for r in range(NR)]
    for r in range(1, NR):
        need_vt[r] = max(need_vt[r], need_vt[r - 1])

    load_w()

    def new_qt_ot(r):
        qt_r = [rot.tile([128, 512], BF16, tag=f"qt{hp}", name=f"qt{hp}_{r}")
                for hp in range(NPAIR)]
        ot_r = [rot.tile([128, 512], BF16, tag=f"ot{hp}", name=f"ot{hp}_{r}",
                         bufs=4)
                for hp in range(NPAIR)]
        return qt_r, ot_r

    fillers = []    # (kind, fn); kind "s" = structural (must emit before
    # the next r's attention), "d" = freely deferrable

    def fill(k):
        # pop structural units first so deferrable work (o-proj, last AV)
        # accumulates for the exp-bound final range
        for _ in range(k):
            if not fillers:
                return
            for i, (kind, fn) in enumerate(fillers):
                if kind == "s":
                    del fillers[i]
                    fn()
                    break
            else:
                fillers.pop(0)[1]()

    def drain_structural():
        # emit every remaining structural unit (anything the next r's
        # emissions depend on must precede them in the PE stream)
        i = 0
        while i < len(fillers):
            kind, fn = fillers[i]
            if kind == "s":
                del fillers[i]
                fn()
            else:
                i += 1

    qt_r, ot_r = new_qt_ot(0)
    for u in qk_units_sc(0, qt_r):
        u()
    for u in v_units(0, need_vt[0]):
        u()

    for r in range(NR):
        nq, no = None, None
        if r + 1 < NR:
            nq, no = new_qt_ot(r + 1)
            fillers.extend(("s", u) for u in qk_units_sc(r + 1, nq))
            fillers.extend(
                ("s", u) for u in v_units(need_vt[r], need_vt[r + 1]))
        pts_by_hp = {}
        for hp in range(NPAIR):
            pts_by_hp[hp] = scores_phase(hp, r, qt_r, fill)
            if hp >= 1:
                av_phase(hp - 1, r, pts_by_hp.pop(hp - 1), ot_r)
                fill(1)
        # last head pair's AV + this r's output projection become fillers
        # for the next r's exp-bound score sections
        fillers.append(("d", lambda hp=NPAIR - 1, r=r,
                        p=pts_by_hp.pop(NPAIR - 1), o=ot_r:
                        av_phase(hp, r, p, o)))
        for t in range(4 * r, 4 * (r + 1)):
            for nh in range(2):
                fillers.append(
                    ("d", lambda t=t, nh=nh, o=ot_r: o_unit(o, t, nh)))
        drain_structural()
        qt_r, ot_r = nq, no
    while fillers:
        fillers.pop(0)[1]()


_CACHE = {}
RUN_WALLS = []
LAST_RESULTS = None


def _get_program(mask_key, live, n_pat):
    if mask_key not in _CACHE:
        _CACHE[mask_key] = build_program(live, n_pat)
    return _CACHE[mask_key]


def kernel(q, k, v, mask, wq, bq, wk, bk, wv, bv, wo, bo):
    q = np.asarray(q, np.float32)
    k = np.asarray(k, np.float32)
    v = np.asarray(v, np.float32)
    mask = np.asarray(mask, bool)
    wq, wk, wv, wo = (np.asarray(w, np.float32) for w in (wq, wk, wv, wo))
    bq, bk, bv, bo = (np.asarray(b, np.float32) for b in (bq, bk, bv, bo))

    live, patterns = classify_mask(mask)
    n_pat = len(patterns)
    nc = _get_program(mask.tobytes(), live, n_pat)

    pats = np.zeros((max(n_pat, 1), 128, 512), BF)
    for i, p in enumerate(patterns):
        pats[i] = p.astype(BF)

    in_maps = []
    for c in range(NCORES):
        b, g = divmod(c, 2)
        gs = slice(DL * g, DL * (g + 1))
        in_maps.append({
            "xqt": np.ascontiguousarray(q[b].T).astype(BF),
            "xkt": np.ascontiguousarray(k[b].T).astype(BF),
            "xvt": np.ascontiguousarray(v[b].T).astype(BF),
            "wqt": np.ascontiguousarray(wq[gs].T).astype(BF),
            "wkt": np.ascontiguousarray(wk[gs].T).astype(BF),
            "wvt": np.ascontiguousarray(wv[gs].T).astype(BF),
            "wot": np.ascontiguousarray(wo[:, gs].T).astype(BF),
            "bqp": np.ascontiguousarray(
                bq[gs].reshape(NPAIR, 128).T).astype(np.float32),
            "bkp": np.ascontiguousarray(
                bk[gs].reshape(NPAIR, 128).T).astype(np.float32),
            "pats": pats,
        })

    import time as _time
    _t0 = _time.time()
    res = run_bass_kernel_spmd(nc, in_maps, core_ids=list(range(NCORES)))
    RUN_WALLS.append(_time.time() - _t0)
    global LAST_RESULTS
    LAST_RESULTS = res

    # host-side: sum the two head-group partials, add bo and the V-bias
    # contribution (softmax rows sum to 1, so + bv @ wo.T exactly)
    bo_full = bo + bv @ wo.T
    out = np.empty((B, S, D), np.float32)
    for b in range(B):
        out[b] = (res.results[2 * b]["outp"] + res.results[2 * b + 1]["outp"]
                  + bo_full)
    return out


# revision 62
# speedup vs baseline: 1.0364x; 1.0364x over previous
"""Multi-head attention (B=4, S=2048, D=1024, H=16) on 8 NeuronCores.

Sharding: core c -> (batch b = c//2, head-group g = c%2 of 8 heads).
Each core computes QKV projections for its 8 heads, causal attention, and a
row-sharded output projection partial; the host sums the two partials per
batch and adds the output bias (with the V-bias contribution bv @ wo.T
folded in on the host, so the device never sees bv).

Device kernel layout (cost-model-driven; matmul cost = out free size):
  * Q/K produced transposed (head-pair dim on partitions); score matmuls
    ST = K @ Q^T contract d_k=64 on partitions, two PE quadrants.
  * Scores/exp computed only on live columns [live0:512) per 128-k block.
  * exp on ScalarE via one strided (p, 2, w) activation per block (both
    heads); multiplicative 0/1 mask patterns (Pool engine) for partial
    blocks; no max-subtraction (logits ~N(0,1)).
  * AV is flipped: out[q;128, V|1;65] = pt[:, qtile]^T @ [V|ones], so the
    streamed free dim is 65 instead of 128 -> ~2x fewer PE cycles, and the
    ones column accumulates the softmax denominator per q on partitions.
  * Normalization = per-partition reciprocal (ACT exp(-ln d) on a strided
    [128, 8] view) + DVE tensor_scalar multiplies; the normalized [q, f]
    tile is PE-transposed back to [f, q] for the output projection.
  * Q/K biases folded into the PSUM evacuation as per-partition adds.
  * Output projection accumulates 4 head-pairs in PSUM and DMAs the fp32
    partial straight to DRAM.
"""

import numpy as np
import ml_dtypes
from contextlib import ExitStack

import concourse.bass as bass
import concourse.bacc as bacc
import concourse.tile as tile
from concourse import mybir
from concourse.bass_utils import run_bass_kernel_spmd
from concourse.masks import make_identity

F32 = mybir.dt.float32
BF16 = mybir.dt.bfloat16
BF = ml_dtypes.bfloat16

B, S, D, H, DK = 4, 2048, 1024, 16, 64
NCORES = 8
GH = 8            # heads per core
DL = GH * DK      # 512 local feature dims
NPAIR = 4         # local head pairs
NR = 4            # q ranges of 512
NKB = S // 128    # 16 k blocks
KTILES = D // 128  # 8 contraction tiles
EXP = mybir.ActivationFunctionType.Exp
LN = mybir.ActivationFunctionType.Ln
SCALE = 1.0 / np.sqrt(DK)


class BlockInfo:
    __slots__ = ("j", "live0", "live1", "pat", "mul0", "mul1")

    def __init__(self, j, live0, live1, pat, mul0, mul1):
        self.j, self.live0, self.live1 = j, live0, live1
        self.pat, self.mul0, self.mul1 = pat, mul0, mul1


def classify_mask(mask):
    """Classify (512 q x 128 k) blocks of the attention mask.

    Returns (live, patterns): live[r] is a list of BlockInfo for the k-blocks
    that have any attendable position; patterns is a list of (128, 512)
    float32 0/1 tiles (k on rows, q-local on cols), deduplicated.
    """
    live = []
    patterns = []
    index = {}
    for r in range(NR):
        row = []
        qs = mask[512 * r: 512 * (r + 1), :]
        for j in range(NKB):
            blk = qs[:, 128 * j: 128 * (j + 1)]       # (512 q, 128 k)
            if not blk.any():
                continue
            if blk.all():
                row.append(BlockInfo(j, 0, 512, None, 0, 0))
                continue
            bt = blk.T                                  # (128 k, 512 q)
            colfull = bt.all(axis=0)
            colany = bt.any(axis=0)
            liveidx = np.nonzero(colany)[0]
            live0, live1 = int(liveidx.min()), int(liveidx.max()) + 1
            nonfull = np.nonzero(~colfull[live0:live1])[0]
            if len(nonfull) == 0:
                row.append(BlockInfo(j, live0, live1, None, 0, 0))
                continue
            mul0 = live0 + int(nonfull.min())
            mul1 = live0 + int(nonfull.max()) + 1
            pat = bt[:, mul0:mul1].astype(np.float32)
            key = (mul1 - mul0, pat.tobytes())
            if key not in index:
                index[key] = len(patterns)
                padded = np.zeros((128, 512), np.float32)
                padded[:, : mul1 - mul0] = pat
                patterns.append(padded)
            row.append(BlockInfo(j, live0, live1, index[key], mul0, mul1))
        if not row:
            raise NotImplementedError(
                "a 512-row q range attends to nothing; fully-masked rows "
                "are not supported"
            )
        for t in range(4):
            if not any(bi.live0 < 128 * (t + 1) and bi.live1 > 128 * t
                       for bi in row):
                raise NotImplementedError(
                    "a 128-row q tile attends to nothing")
        live.append(row)
    if len(patterns) > 8:
        raise NotImplementedError(f"{len(patterns)} unique mask patterns")
    return live, patterns


def build_program(live, n_pat):
    nc = bacc.Bacc("TRN2", target_bir_lowering=False, debug=False,
                   num_devices=NCORES)

    xqt = nc.dram_tensor("xqt", [D, S], BF16, kind="ExternalInput").ap()
    xkt = nc.dram_tensor("xkt", [D, S], BF16, kind="ExternalInput").ap()
    xvt = nc.dram_tensor("xvt", [D, S], BF16, kind="ExternalInput").ap()
    wqt = nc.dram_tensor("wqt", [D, DL], BF16, kind="ExternalInput").ap()
    wkt = nc.dram_tensor("wkt", [D, DL], BF16, kind="ExternalInput").ap()
    wvt = nc.dram_tensor("wvt", [D, DL], BF16, kind="ExternalInput").ap()
    wot = nc.dram_tensor("wot", [DL, D], BF16, kind="ExternalInput").ap()
    # q/k biases pre-transposed on the host: [128 partition dims, NPAIR]
    bqp = nc.dram_tensor("bqp", [128, NPAIR], F32, kind="ExternalInput").ap()
    bkp = nc.dram_tensor("bkp", [128, NPAIR], F32, kind="ExternalInput").ap()
    patd = nc.dram_tensor("pats", [max(n_pat, 1), 128, 512], BF16,
                          kind="ExternalInput").ap()
    outp = nc.dram_tensor("outp", [S, D], F32, kind="ExternalOutput").ap()

    with tile.TileContext(nc) as tc, ExitStack() as ctx:
        emit(ctx, tc, nc, live, n_pat,
             xqt, xkt, xvt, wqt, wkt, wvt, wot, bqp, bkp, patd, outp)
    nc.compile()
    return nc


I32 = mybir.dt.int32
# fp32 fast-reciprocal seed in negative space:
# yn0_bits = (d_bits ^ -1) + RECIP_C2  ==  -(0x7EF311C3 - d_bits)
RECIP_C2 = -17624636


def emit(ctx, tc, nc, live, n_pat,
         xqt, xkt, xvt, wqt, wkt, wvt, wot, bqp_d, bkp_d, patd, outp):
    wpool = ctx.enter_context(tc.tile_pool(name="w", bufs=1))
    qkpool = ctx.enter_context(tc.tile_pool(name="qk", bufs=1))
    vpool = ctx.enter_context(tc.tile_pool(name="vp", bufs=1))
    rot = ctx.enter_context(tc.tile_pool(name="rot", bufs=3))
    xs = ctx.enter_context(tc.tile_pool(name="xs", bufs=2))
    ptp = ctx.enter_context(tc.tile_pool(name="ptp", bufs=38))
    anup = ctx.enter_context(tc.tile_pool(name="anup", bufs=4))
    ntp = ctx.enter_context(tc.tile_pool(name="ntp", bufs=8))
    outs = ctx.enter_context(tc.tile_pool(name="outs", bufs=2))

    stps = ctx.enter_context(tc.tile_pool(name="stps", bufs=2, space="PSUM"))
    avps = ctx.enter_context(tc.tile_pool(name="avps", bufs=2, space="PSUM"))
    pps = ctx.enter_context(tc.tile_pool(name="pps", bufs=2, space="PSUM"))

    # ---- resident tiles ----
    def load(name, dram, shape):
        t = wpool.tile(shape, BF16, tag=name, name=name)
        nc.gpsimd.dma_start(t[:], dram)
        return t

    bqp = wpool.tile([128, NPAIR], F32, tag="bqp", name="bqp")
    nc.gpsimd.dma_start(bqp[:], bqp_d)
    bkp = wpool.tile([128, NPAIR], F32, tag="bkp", name="bkp")
    nc.gpsimd.dma_start(bkp[:], bkp_d)
    pat_sb = [load(f"pat{i}", patd[i], [128, 512]) for i in range(n_pat)]
    ident = wpool.tile([128, 128], BF16, tag="ident")
    make_identity(nc, ident[:])

    def alloc(name, shape):
        return wpool.tile(shape, BF16, tag=name, name=name)

    wq_t = [alloc(f"wq{i}", [128, DL]) for i in range(KTILES)]
    wk_t = [alloc(f"wk{i}", [128, DL]) for i in range(KTILES)]
    wv_t = [alloc(f"wv{i}", [128, DL]) for i in range(KTILES)]
    wo_t = [alloc(f"wo{i}", [128, 512]) for i in range(2 * NPAIR)]

    def load_w():
        for i in range(KTILES):
            nc.gpsimd.dma_start(wq_t[i][:], wqt[128 * i:128 * (i + 1), :])
            nc.gpsimd.dma_start(wk_t[i][:], wkt[128 * i:128 * (i + 1), :])
            nc.gpsimd.dma_start(wv_t[i][:], wvt[128 * i:128 * (i + 1), :])
        for i in range(2 * NPAIR):
            nc.gpsimd.dma_start(
                wo_t[i][:], wot[128 * (i // 2):128 * (i // 2 + 1),
                                512 * (i % 2):512 * (i % 2 + 1)])

    # kt is fully resident (scores for range r read k columns 0..512(r+1));
    # qt/ot only hold the current 512-q range and rotate (bufs=2)
    kt_t = [qkpool.tile([128, S], BF16, tag=f"kt{hp}", name=f"kt{hp}")
            for hp in range(NPAIR)]
    va_t = [vpool.tile([128, GH * 65], BF16, tag=f"va{t}", name=f"va{t}")
            for t in range(NKB)]

    # ---- PE filler units (emitted between score blocks to fill the gaps
    # the in-order PE stream would otherwise spend stalled on exp) ----

    # Q/K projection of one (tensor, head pair) for 512-token chunk sc
    def qk_unit(x_c, w_t, b_sb, hp, dest, sc):
        ps = pps.tile([128, 512], F32, tag="pps", name="qkps")
        for kt in range(KTILES):
            nc.tensor.matmul(
                ps[:], w_t[kt][:, 128 * hp:128 * (hp + 1)],
                x_c[kt][:], start=(kt == 0), stop=(kt == KTILES - 1))
        if dest is not None:
            nc.vector.tensor_scalar_add(dest[:], ps[:], b_sb[:, hp:hp + 1])
        else:
            nc.vector.tensor_scalar_add(
                kt_t[hp][:, 512 * sc:512 * (sc + 1)], ps[:],
                b_sb[:, hp:hp + 1])

    def qk_units_sc(sc, qt_r):
        # sc0 is on the critical startup path: spread its loads over the SP
        # and (still idle) ACT DMA queues
        kq = nc.scalar if sc == 0 else nc.sync
        xq_c, xk_c = [], []
        for i in range(KTILES):
            xq = xs.tile([128, 512], BF16, tag=f"xq{i}", name=f"xq{i}_{sc}", bufs=1)
            nc.sync.dma_start(
                xq[:], xqt[128 * i:128 * (i + 1), 512 * sc:512 * (sc + 1)])
            xq_c.append(xq)
            xk = xs.tile([128, 512], BF16, tag=f"xk{i}", name=f"xk{i}_{sc}", bufs=1)
            kq.dma_start(
                xk[:], xkt[128 * i:128 * (i + 1), 512 * sc:512 * (sc + 1)])
            xk_c.append(xk)
        units = []
        for hp in range(NPAIR):
            units.append(lambda hp=hp: qk_unit(
                xq_c, wq_t, bqp, hp, qt_r[hp][:], sc))
            units.append(lambda hp=hp: qk_unit(
                xk_c, wk_t, bkp, hp, None, sc))
        return units

    # V projection of one 128-token tile (all 8 heads), ones-augmented
    def v_units(t0, t1):
        if t0 >= t1:
            return []
        assert t1 - t0 <= 4
        chunks = []
        for kt in range(KTILES):
            xt = xs.tile([128, 512], BF16, tag="xv", name=f"xv{t0}_{kt}",
                         bufs=9)
            nc.gpsimd.dma_start(
                xt[:, 0:128 * (t1 - t0)],
                xvt[128 * kt:128 * (kt + 1), 128 * t0:128 * t1])
            chunks.append(xt)

        def v_unit(t):
            o = 128 * (t - t0)
            ps = pps.tile([128, 512], F32, tag="pps", name="vps")
            for kt in range(KTILES):
                nc.tensor.matmul(ps[:], chunks[kt][:, o:o + 128],
                                 wv_t[kt][:], start=(kt == 0),
                                 stop=(kt == KTILES - 1))
            va = va_t[t].rearrange("p (h w) -> p h w", w=65)
            nc.vector.tensor_copy(
                va[:, :, 0:64], ps.rearrange("p (h w) -> p h w", w=64))
            nc.gpsimd.memset(va[:, :, 64:65], 1.0)

        return [lambda t=t: v_unit(t) for t in range(t0, t1)]

    # output projection partial for one (q-tile, 512-col half)
    def o_unit(ot_r, t, nh):
        ps = pps.tile([128, 512], F32, tag="pps", name="ops")
        for hp in range(NPAIR):
            nc.tensor.matmul(
                ps[:], ot_r[hp][:, 128 * (t % 4):128 * (t % 4 + 1)],
                wo_t[2 * hp + nh][:],
                start=(hp == 0), stop=(hp == NPAIR - 1))
        osb = outs.tile([128, 512], F32, tag="osb")
        nc.vector.tensor_copy(osb[:], ps[:])
        nc.sync.dma_start(
            outp[128 * t:128 * (t + 1), 512 * nh:512 * (nh + 1)], osb[:])

    # ---- attention phases ----
    def scores_phase(hp, r, qt_r, fill):
        qt, kt_ = qt_r[hp], kt_t[hp]
        pts = []
        for bi in live[r]:
            j, lo, hi = bi.j, bi.live0, bi.live1
            st = stps.tile([128, 1024], F32, tag="st")
            nc.tensor.matmul(
                st[:, lo:hi],
                kt_[0:64, 128 * j:128 * (j + 1)],
                qt[0:64, lo:hi],
                start=True, stop=True, tile_position=(0, 0))
            nc.tensor.matmul(
                st[:, 512 + lo:512 + hi],
                kt_[64:128, 128 * j:128 * (j + 1)],
                qt[64:128, lo:hi],
                start=True, stop=True, tile_position=(64, 0))
            pt = ptp.tile([128, 1024], BF16, tag="pt")
            st3 = st.rearrange("p (h w) -> p h w", w=512)
            pt3 = pt.rearrange("p (h w) -> p h w", w=512)
            nc.scalar.activation(pt3[:, :, lo:hi], st3[:, :, lo:hi],
                                 EXP, scale=float(SCALE))
            # zero dead columns inside covered 128-q tiles so the flipped AV
            # matmuls never read garbage (no-op for causal masks)
            plo, phi = (lo // 128) * 128, -(-hi // 128) * 128
            for h in range(2):
                if bi.pat is not None:
                    sl = pt[:, 512 * h + bi.mul0:512 * h + bi.mul1]
                    nc.gpsimd.tensor_mul(
                        sl, sl, pat_sb[bi.pat][:, 0:bi.mul1 - bi.mul0])
                if plo < lo:
                    nc.gpsimd.memset(pt[:, 512 * h + plo:512 * h + lo], 0.0)
                if hi < phi:
                    nc.gpsimd.memset(pt[:, 512 * h + hi:512 * h + phi], 0.0)
            pts.append((bi, pt, 0))
            if len(pts) % CADL[r] == 0:
                fill(1)
        return pts

    def emit_recip(danu, dw3, lo, hi):
        # negative-space fast reciprocal of d columns [lo:hi):
        # bit-trick seed then 2 Newton iterations yn' = yn*(2 + d*yn),
        # final sign-bit flip -> +1/d. Plain DVE int/float ops only.
        dv, yn3, m3 = danu[:, lo:hi, :], dw3[:, lo:hi, :], dw3[:, 8 + lo:8 + hi, :]
        nc.vector.tensor_scalar(
            out=m3.bitcast(I32), in0=dv.bitcast(I32),
            scalar1=-1, scalar2=None, op0=mybir.AluOpType.bitwise_xor)
        nc.vector.tensor_scalar(
            out=yn3.bitcast(I32), in0=m3.bitcast(I32),
            scalar1=RECIP_C2, scalar2=None, op0=mybir.AluOpType.add)
        for _ in range(2):
            nc.vector.tensor_mul(m3, dv, yn3)
            nc.vector.scalar_tensor_tensor(
                out=yn3, in0=m3, scalar=2.0, in1=yn3,
                op0=mybir.AluOpType.add, op1=mybir.AluOpType.mult)
        nc.vector.tensor_scalar(
            out=yn3.bitcast(I32), in0=yn3.bitcast(I32),
            scalar1=-(1 << 31), scalar2=None,
            op0=mybir.AluOpType.bitwise_xor)

    def av_units(hp, r, pts, ot_r, after_t=None):
        # flipped AV per 128-q tile: out [q, V|1] accumulated over k blocks.
        # Returns 4 per-q-tile closures so the work can be woven into the
        # next score section's exp-stall gaps. Each tile finishes with its
        # own reciprocal + normalize + transpose (+ optional tail hook).
        cell = {}

        def sweep(t):
            if not cell:
                cell["anu"] = anup.tile([128, 520], F32, tag="anu",
                                        name=f"anu{hp}_{r}")
                cell["dw"] = rot.tile([128, 16], F32, tag="dw",
                                      name=f"dw{hp}_{r}")
            anu, dw = cell["anu"], cell["dw"]
            danu = anu.rearrange("p (x w) -> p x w", w=65)[:, :, 64:65]
            dw3 = dw.rearrange("p (x w) -> p x w", w=1)
            js_t = [(bi, pt, base) for bi, pt, base in pts
                    if bi.live0 < 128 * (t + 1) and bi.live1 > 128 * t]
            nj = len(js_t)
            for h in range(2):
                # own bank per accumulation group (start zeroes a whole bank)
                av = avps.tile([128, 65], F32, tag="av")
                hl = 2 * hp + h
                for ji, (bi, pt, base) in enumerate(js_t):
                    o0 = base + 512 * h + 128 * t
                    nc.tensor.matmul(
                        av[:], pt[:, o0:o0 + 128],
                        va_t[bi.j][:, 65 * hl:65 * (hl + 1)],
                        start=(ji == 0), stop=(ji == nj - 1))
                o = 65 * (2 * t + h)
                nc.vector.tensor_copy(anu[:, o:o + 65], av[:])
            # reciprocal right after the sweep: its DVE chain drains while
            # other PE work (scores, fillers) runs, so the deferred norm's
            # transpose never stalls on it
            emit_recip(danu, dw3, 2 * t, 2 * t + 2)

        def norm(t):
            anu, dw = cell["anu"], cell["dw"]
            nt = ntp.tile([128, 128], BF16, tag="nt")
            for h in range(2):
                o = 65 * (2 * t + h)
                nc.vector.tensor_scalar_mul(
                    nt[:, 64 * h:64 * (h + 1)], anu[:, o:o + 64],
                    dw[:, 2 * t + h:2 * t + h + 1])
            tp = pps.tile([128, 128], BF16, tag="pps", name="tp")
            nc.tensor.transpose(tp[:], nt[:], ident[:])
            nc.vector.tensor_copy(
                ot_r[hp][:, 128 * t:128 * (t + 1)], tp[:])
            if after_t is not None:
                after_t(t)

        def unit(t):
            if t > 0:
                norm(t - 1)
            sweep(t)

        return [lambda t=t: unit(t) for t in range(4)] + [lambda: norm(3)]

    # ---- schedule: r-outer, hp-inner; qk(r+1)/v(r+1)/o-proj(r-1) are
    # emitted as fillers inside the score sections ----
    need_vt = [max(bi.j for bi in live[r]) + 1 for r in range(NR)]
    for r in range(1, NR):
        need_vt[r] = max(need_vt[r], need_vt[r - 1])

    load_w()

    def new_qt_ot(r):
        qt_r = [rot.tile([128, 512], BF16, tag=f"qt{hp}", name=f"qt{hp}_{r}")
                for hp in range(NPAIR)]
        ot_r = [rot.tile([128, 512], BF16, tag=f"ot{hp}", name=f"ot{hp}_{r}",
                         bufs=3)
                for hp in range(NPAIR)]
        return qt_r, ot_r

    import os as _os
    PRIO = _os.environ.get("SCHED_PRIO", "0") == "1"
    _cad = _os.environ.get("SCHED_CAD", "2,3,2,4")
    CADL = ([int(x) for x in _cad.split(",")] * 4)[:4] \
        if "," in _cad else [int(_cad)] * 4
    AVFILL = int(_os.environ.get("SCHED_AVFILL", "2"))

    fillers = []    # (kind, pid, fn); kind "s" = structural (must emit
    # before the next r's attention), "a" = AV unit with phase id pid
    # (must emit within two phases, before its pt tiles are recycled),
    # "d" = freely deferrable

    def fill(k):
        for _ in range(k):
            if not fillers:
                return
            fillers.pop(0)[2]()

    def drain_structural():
        # emit every remaining structural unit (anything the next r's
        # emissions depend on must precede them in the PE stream)
        i = 0
        while i < len(fillers):
            kind, pid, fn = fillers[i]
            if kind == "s":
                del fillers[i]
                fn()
            else:
                i += 1

    def drain_av(pid_max):
        # emit AV units at or below pid_max so their pt tiles can recycle
        # before the score sections that need the slots are emitted
        i = 0
        while i < len(fillers):
            kind, pid, fn = fillers[i]
            if kind == "a" and pid <= pid_max:
                del fillers[i]
                fn()
            else:
                i += 1

    qt_r, ot_r = new_qt_ot(0)
    for u in qk_units_sc(0, qt_r):
        u()
    for u in v_units(0, need_vt[0]):
        u()

    for r in range(NR):
        nq, no = None, None
        if r + 1 < NR:
            nq, no = new_qt_ot(r + 1)
            fillers.extend(("s", None, u)
                           for u in qk_units_sc(r + 1, nq))
            fillers.extend(("s", None, u)
                           for u in v_units(need_vt[r], need_vt[r + 1]))
        pts_by_hp = {}
        for hp in range(NPAIR):
            if hp >= 1:
                # previous pair's AV weaves into this section's gaps
                fillers.extend(
                    ("a", 4 * r + hp - 1, u) for u in av_units(
                        hp - 1, r, pts_by_hp.pop(hp - 1), ot_r))
            drain_av(4 * r + hp - 2)
            pts_by_hp[hp] = scores_phase(hp, r, qt_r, fill)
        if r < NR - 1:
            # last head pair's AV + this r's output projection become
            # fillers for the next r's exp-bound score sections
            fillers.extend(
                ("a", 4 * r + NPAIR - 1, u) for u in av_units(
                    NPAIR - 1, r, pts_by_hp.pop(NPAIR - 1), ot_r))
            for t in range(4 * r, 4 * (r + 1)):
                for nh in range(2):
                    fillers.append(
                        ("d", None,
                         lambda t=t, nh=nh, o=ot_r: o_unit(o, t, nh)))
            drain_structural()
            qt_r, ot_r = nq, no
        else:
            # final range: drain remaining fillers, then run the last AV
            # with the output projection interleaved per q-tile
            while fillers:
                fillers.pop(0)[2]()

            def tail_o(t, o=ot_r, r=r):
                for nh in range(2):
                    o_unit(o, 4 * r + t, nh)

            for u in av_units(NPAIR - 1, r, pts_by_hp.pop(NPAIR - 1), ot_r,
                              after_t=tail_o):
                u()
    while fillers:
        fillers.pop(0)[2]()


_CACHE = {}
RUN_WALLS = []
LAST_RESULTS = None


def _get_program(mask_key, live, n_pat):
    if mask_key not in _CACHE:
        _CACHE[mask_key] = build_program(live, n_pat)
    return _CACHE[mask_key]


def kernel(q, k, v, mask, wq, bq, wk, bk, wv, bv, wo, bo):
    q = np.asarray(q, np.float32)
    k = np.asarray(k, np.float32)
    v = np.asarray(v, np.float32)
    mask = np.asarray(mask, bool)
    wq, wk, wv, wo = (np.asarray(w, np.float32) for w in (wq, wk, wv, wo))
    bq, bk, bv, bo = (np.asarray(b, np.float32) for b in (bq, bk, bv, bo))

    live, patterns = classify_mask(mask)
    n_pat = len(patterns)
    nc = _get_program(mask.tobytes(), live, n_pat)

    pats = np.zeros((max(n_pat, 1), 128, 512), BF)
    for i, p in enumerate(patterns):
        pats[i] = p.astype(BF)

    in_maps = []
    for c in range(NCORES):
        b, g = divmod(c, 2)
        gs = slice(DL * g, DL * (g + 1))
        in_maps.append({
            "xqt": np.ascontiguousarray(q[b].T).astype(BF),
            "xkt": np.ascontiguousarray(k[b].T).astype(BF),
            "xvt": np.ascontiguousarray(v[b].T).astype(BF),
            "wqt": np.ascontiguousarray(wq[gs].T).astype(BF),
            "wkt": np.ascontiguousarray(wk[gs].T).astype(BF),
            "wvt": np.ascontiguousarray(wv[gs].T).astype(BF),
            "wot": np.ascontiguousarray(wo[:, gs].T).astype(BF),
            "bqp": np.ascontiguousarray(
                bq[gs].reshape(NPAIR, 128).T).astype(np.float32),
            "bkp": np.ascontiguousarray(
                bk[gs].reshape(NPAIR, 128).T).astype(np.float32),
            "pats": pats,
        })

    import time as _time
    _t0 = _time.time()
    res = run_bass_kernel_spmd(nc, in_maps, core_ids=list(range(NCORES)))
    RUN_WALLS.append(_time.time() - _t0)
    global LAST_RESULTS
    LAST_RESULTS = res

    # host-side: sum the two head-group partials, add bo and the V-bias
    # contribution (softmax rows sum to 1, so + bv @ wo.T exactly)
    bo_full = bo + bv @ wo.T
    out = np.empty((B, S, D), np.float32)
    for b in range(B):
        out[b] = (res.results[2 * b]["outp"] + res.results[2 * b + 1]["outp"]
                  + bo_full)
    return out
